# revision 19
# baseline (speedup 1.0000x reference)
"""GCN layer (scale + segment-sum + linear + relu) on 8 TRN2 cores.

Sharding: each core owns a contiguous range of 6250 dst nodes and processes
every edge pointing into that range (same dst-sharding as a gather design),
but the per-edge data movement is INVERTED: instead of gathering a 256 B
row per edge (22.8 ns/descriptor), the kernel streams the feature table in
sequentially once, scales it by rsqrt(out_deg) on DVE, and scatter-adds the
96 B message of each edge into a DRAM accumulator via dma_scatter_add
(8.5 ns/descriptor — elem 48 fp16 with a 256 B row stride).

The SDMA CCE read-modify-write is NOT atomic across engines, so two adds to
the same accumulator row inside one scatter instruction (or in two
concurrently draining instructions) lose updates. The host therefore packs
edges into batches where (a) the sources form one slot per table position
over a contiguous slice (holes encoded as -1 indices, skipped positionally
by the Q7 ucode), and (b) the dst rows are DISTINCT — via per-batch maximum
bipartite matching. Batches rotate over 3 independent accumulators so their
WAW chains pipeline without racing; the tail transpose-reads the three
accumulators back, sums them, applies rsqrt(in_deg), and runs the 48x48
linear + relu exactly like the tail of the gather design.

The feature table is permuted per core by in-core degree (descending) so a
round-r batch touches only a prefix of positions; all value math (rsqrt,
scaling, sums, linear) runs on device — host work is index/format only.
One program is compiled for all 8 cores: batch shapes are the max over
cores, and per-core variation lives entirely in the index data (-1 pads).
"""

import numpy as np

N = 50000
E = 1600000
D = 48
NCORES = 8
NPC = 6250             # dst nodes per core
BLOCKS = 49            # dst range padded to 49*128 = 6272
NPAD = BLOCKS * 128
TCH = 391              # feature-table chunks: 391*128 = 50048 positions
NTAB = TCH * 128
SLCH = 36              # slice = 36 chunks = 4608 positions
SL = SLCH * 128
NACC = 3               # rotating DRAM accumulators

_CACHE = {}


# ---------------------------------------------------------------------------
# Host-side preprocessing: edge batching (bipartite matching per batch so
# every dst row within a scatter instruction is unique), node permutation,
# index wrapping. All value math runs on device.
# ---------------------------------------------------------------------------

NBINS = 8              # matched batches per slice in the main sweep
THETA = 0.5            # natural-density gate for emitting a (slice, bin)


def _build_core_batches(src_c, dstl_c):
    """Pack one core's edges into race-free scatter batches.

    Main sweep: per 36-chunk slice, up to NBINS maximum matchings; each
    matched set becomes one batch (distinct dst rows by construction).
    Everything left over goes to the annex: each remaining edge gets a
    fresh dedicated table slot (its source row duplicated), grouped by
    rank-within-dst so every annex batch is dense and dst-distinct.

    Returns (pi, main: {(s, b): (lo, hi, idxarr)}, annex_groups:
    [dst arrays], annex_srcs: [src-node arrays]) with slot arrays indexed
    from the slice base.
    """
    from scipy.sparse import csr_matrix
    from scipy.sparse.csgraph import maximum_bipartite_matching

    degc = np.bincount(src_c, minlength=N)
    pi = np.argsort(-degc, kind="stable")
    pos = np.empty(N, np.int64)
    pos[pi] = np.arange(N)

    p = pos[src_c]
    order = np.argsort(p, kind="stable")
    ps = p[order]
    ds = dstl_c[order].astype(np.int64)
    src_o = src_c[order]
    ne = len(ps)
    consumed = np.zeros(ne, bool)
    cnt = np.bincount(ps, minlength=NTAB)
    indptr = np.zeros(NTAB + 1, np.int64)
    indptr[1:] = np.cumsum(cnt)
    degpos = cnt.copy()

    main = {}
    for s in range(-(-NTAB // SL)):
        base = s * SL
        hi = min(base + SL, NTAB)
        e0, e1 = indptr[base], indptr[hi]
        if e0 == e1:
            continue
        for b in range(NBINS):
            nat = int((degpos[base:hi] > b).sum())
            if nat < THETA * (hi - base):
                break
            sub = np.arange(e0, e1)[~consumed[e0:e1]]
            if sub.size == 0:
                break
            rows = ps[sub] - base
            cols = ds[sub]
            g = csr_matrix((np.ones(sub.size, np.int8), (rows, cols)),
                           shape=(hi - base, NPC))
            m = maximum_bipartite_matching(g, perm_type="column")
            mr = np.flatnonzero(m >= 0)
            if mr.size == 0:
                break
            okey = rows * NPC + cols
            osort = np.argsort(okey, kind="stable")
            want = mr * NPC + m[mr]
            j = np.searchsorted(okey[osort], want)
            pick = sub[osort[j]]
            consumed[pick] = True
            arr = np.full(hi - base, -1, np.int16)
            arr[mr] = m[mr].astype(np.int16)
            main[(s, b)] = (int(mr[0]), int(mr[-1]) + 1, arr)

    rem = np.flatnonzero(~consumed)
    annex_groups = []
    annex_srcs = []
    if rem.size:
        rd = ds[rem]
        rs = src_o[rem]
        o2 = np.argsort(rd, kind="stable")
        rds = rd[o2]
        rss = rs[o2]
        runstart = np.r_[0, np.flatnonzero(np.diff(rds)) + 1]
        runid = np.zeros(rem.size, np.int64)
        runid[runstart] = 1
        runid = np.cumsum(runid) - 1
        rank = np.arange(rem.size) - runstart[runid]
        for k in range(int(rank.max()) + 1):
            selk = rank == k
            annex_groups.append(rds[selk].astype(np.int16))
            annex_srcs.append(rss[selk])
    return pi, main, annex_groups, annex_srcs


def _host_prep(features, src, dst):
    src = np.asarray(src).astype(np.int64)
    dst = np.asarray(dst).astype(np.int64)
    feats = np.asarray(features, dtype=np.float32)
    feats16 = feats.astype(np.float16)

    out_deg = np.bincount(src, minlength=N).astype(np.int64)
    in_deg = np.bincount(dst, minlength=N).astype(np.int64)
    iso = bool((in_deg == 0).any())

    core = dst // NPC
    raw = []
    for c in range(NCORES):
        sel = core == c
        raw.append(_build_core_batches(src[sel], dst[sel] - c * NPC))

    # ---- uniform cross-core plan -------------------------------------
    # main batches: union of (slice, bin) keys; per batch the chunk start
    # and length cover every core's matched span
    keys = sorted({k for _, m, _, _ in raw for k in m})
    mplan = []
    for (s, b) in keys:
        base = s * SL
        lo = SL
        hiv = 0
        for _, m, _, _ in raw:
            ent = m.get((s, b))
            if ent is not None:
                lo = min(lo, ent[0])
                hiv = max(hiv, ent[1])
        ca = s * SLCH + lo // 128
        L = -(-(hiv - (lo // 128) * 128) // 16) * 16
        mplan.append((s, b, ca, L))

    # annex: rank-group k starts at a uniform chunk; its slot count is the
    # max over cores (padded to 16)
    ngroups = max(len(g) for _, _, g, _ in raw)
    gch = []
    for k in range(ngroups):
        gmax = max((len(g[k]) if k < len(g) else 0) for _, _, g, _ in raw)
        gch.append(-(-gmax // 128))
    astart = []
    ach = TCH
    for k in range(ngroups):
        astart.append(ach)
        ach += gch[k]
    tch2 = ach
    aplan = []
    for k in range(ngroups):
        gmax = max((len(g[k]) if k < len(g) else 0) for _, _, g, _ in raw)
        aplan.append((astart[k], -(-gmax // 16) * 16))

    # combined device plan: (chunk_start, padded_len) per batch, in
    # slice-major order so feature-slice loads stay just ahead of use;
    # annex batches run last (their table chunks load mid-phase)
    plan_list = [(ca, L) for _, _, ca, L in mplan] + aplan
    order = list(range(len(plan_list)))
    plan = tuple(plan_list[i] for i in order)

    per_core = []
    for c in range(NCORES):
        pi, main, groups, gsrcs = raw[c]
        featp = np.zeros((tch2 * 128, D), np.float16)
        featp[:N] = feats16[pi]
        degp = np.zeros(tch2 * 128, np.float16)
        degp[:N] = out_deg[pi]
        for k in range(len(groups)):
            a0 = astart[k] * 128
            featp[a0:a0 + len(gsrcs[k])] = feats16[gsrcs[k]]
            degp[a0:a0 + len(gsrcs[k])] = out_deg[gsrcs[k]]
        featp_dev = np.ascontiguousarray(
            featp.reshape(tch2, 128, D).transpose(1, 0, 2))
        degp_dev = np.ascontiguousarray(degp.reshape(tch2, 128).T)

        bufs = []
        for (s, b, ca, L) in mplan:
            buf = np.full(L, -1, np.int16)
            ent = main.get((s, b))
            if ent is not None:
                off = ca - s * SLCH
                seg = ent[2][off * 128: off * 128 + L]
                buf[:len(seg)] = seg
            bufs.append(buf)
        for k, (cak, L) in enumerate(aplan):
            buf = np.full(L, -1, np.int16)
            if k < len(groups):
                buf[:len(groups[k])] = groups[k]
            bufs.append(buf)
        # the Q7 scatter ucode wedges on long trailing -1 runs across many
        # instructions; point trailing pads at the accumulator's unused pad
        # rows [NPC, NPAD) instead (their garbage is discarded by the tail)
        for buf in bufs:
            v = np.flatnonzero(buf >= 0)
            last = int(v[-1]) + 1 if v.size else 0
            t = np.arange(len(buf) - last)
            buf[last:] = (NPC + t % (NPAD - NPC)).astype(np.int16)
        bufs = [bufs[i] for i in order]
        chunks = []
        for buf in bufs:
            L = len(buf)
            wrapped = buf.reshape(L // 16, 16).T
            chunks.append(np.broadcast_to(
                wrapped[None, :, :], (8, 16, L // 16)).reshape(128, L // 16))
        idxcat = np.ascontiguousarray(np.concatenate(chunks, axis=1))

        nlo = c * NPC
        ind = np.zeros(NPAD, np.uint8)
        ind[:NPC] = np.minimum(in_deg[nlo:nlo + NPC], 255)
        indegT_dev = np.ascontiguousarray(
            np.broadcast_to(ind[None, :], (D, NPAD)))

        entry = dict(featp=featp_dev, degp=degp_dev, idxcat=idxcat,
                     indegT=indegT_dev)
        if iso:
            fc = np.zeros((NPAD, D), np.float32)
            fc[:NPC] = feats[nlo:nlo + NPC]
            entry["featT"] = np.ascontiguousarray(fc.T)
        per_core.append(entry)

    assert in_deg.max() <= 255
    tot16 = sum(L // 16 for _, L in plan)
    meta = dict(plan=plan, tot16=tot16, iso=iso, tch2=tch2)
    return per_core, meta


# ---------------------------------------------------------------------------
# Device program
# ---------------------------------------------------------------------------

def _build_program(meta):
    import concourse.tile as tile
    from concourse import bacc, mybir

    plan = meta["plan"]
    tot16 = meta["tot16"]
    iso = meta["iso"]
    tch2 = meta["tch2"]
    f16 = mybir.dt.float16
    f32 = mybir.dt.float32
    i16 = mybir.dt.int16
    u8 = mybir.dt.uint8
    AF = mybir.ActivationFunctionType
    OP = mybir.AluOpType

    nc = bacc.Bacc("TRN2", target_bir_lowering=False, debug=False,
                   num_devices=NCORES, num_swdge_queues=2,
                   dynamic_dma_scratch_size=49152)

    featpD = nc.dram_tensor("featp", [128, tch2, D], f16,
                            kind="ExternalInput").ap()
    degpD = nc.dram_tensor("degp", [128, tch2], f16, kind="ExternalInput").ap()
    idxcatD = nc.dram_tensor("idxcat", [128, tot16], i16,
                             kind="ExternalInput").ap()
    indegTD = nc.dram_tensor("indegT", [D, NPAD], u8, kind="ExternalInput").ap()
    wbD = nc.dram_tensor("wb", [D, D], f16, kind="ExternalInput").ap()
    biasD = nc.dram_tensor("bias", [D, 1], f32, kind="ExternalInput").ap()
    if iso:
        featTD = nc.dram_tensor("featT", [D, NPAD], f32,
                                kind="ExternalInput").ap()
    accD = [nc.dram_tensor(f"acc{a}", [NPAD, 128], f16,
                           kind="ExternalOutput").ap()
            for a in range(NACC)]
    outD = nc.dram_tensor("out", [D, NPAD], f16, kind="ExternalOutput").ap()

    with tile.TileContext(nc) as tc:
        with tc.tile_pool(name="const", bufs=1) as cpool, \
             tc.tile_pool(name="big", bufs=1) as bigpool:

            wb_s = cpool.tile([D, D], f16, tag="wb")
            nc.scalar.dma_start(out=wb_s[:], in_=wbD)
            bias_s = cpool.tile([D, 1], f32, tag="bias")
            nc.scalar.dma_start(out=bias_s[:], in_=biasD)

            # zero the accumulators first so the scatters can start early
            z = bigpool.tile([128, 25, 128], f16, tag="z")
            nc.vector.memset(z[:], 0.0)
            for a in range(NACC):
                av = accD[a].rearrange("(b p) c -> p b c", p=128)
                nc.sync.dma_start(out=av[:, 0:25, :], in_=z[:])
                nc.sync.dma_start(out=av[:, 25:BLOCKS, :], in_=z[:, 0:24, :])

            # per-src scale ci = rsqrt(max(out_deg, 1)); degrees are exact
            # small ints in fp16
            deg_s = bigpool.tile([128, tch2], f16, tag="deg")
            nc.scalar.dma_start(out=deg_s[:], in_=degpD)
            ci = bigpool.tile([128, tch2], f16, tag="ci")

            # resident idx data on the otherwise-idle ACT queue, split into
            # 4 tiles so early batches only wait on the first small load
            ngrp = 4
            goff = [0]
            gsz = []
            per = -(-len(plan) // ngrp)
            bnd = []
            o = 0
            for gi in range(ngrp):
                lo_b = gi * per
                hi_b = min((gi + 1) * per, len(plan))
                w = sum(L // 16 for _, L in plan[lo_b:hi_b])
                bnd.append((lo_b, hi_b, o))
                gsz.append(w)
                o += w
                goff.append(o)
            idx_tiles = []
            for gi in range(ngrp):
                t = bigpool.tile([128, max(gsz[gi], 1)], i16, tag=f"idx{gi}")
                if gsz[gi]:
                    nc.scalar.dma_start(
                        out=t[:],
                        in_=idxcatD[:, goff[gi]:goff[gi] + gsz[gi]])
                idx_tiles.append(t)

            # feature table: load + scale per 44-chunk slice, ordered by the
            # first batch that needs each slice so early batches start fast
            featp_s = bigpool.tile([128, tch2, D], f16, tag="featp")
            LCH = SLCH
            nslice = -(-tch2 // LCH)
            first_use = [len(plan)] * nslice
            for i, (ca, L) in enumerate(plan):
                K = -(-L // 128)
                for s in range(ca // LCH, min((ca + K - 1) // LCH + 1,
                                              nslice)):
                    first_use[s] = min(first_use[s], i)
            sorder = sorted(range(nslice), key=lambda s: first_use[s])
            with tc.tile_pool(name="cip", bufs=3) as cipool:
                for s in sorder:
                    ca = s * LCH
                    kch = min(LCH, tch2 - ca)
                    nc.sync.dma_start(out=featp_s[:, ca:ca + kch, :],
                                      in_=featpD[:, ca:ca + kch, :])
                    c32 = cipool.tile([128, LCH], f32, tag="c32")
                    nc.vector.tensor_scalar_max(c32[:, 0:kch],
                                                deg_s[:, ca:ca + kch], 1.0)
                    nc.scalar.activation(c32[:, 0:kch], c32[:, 0:kch],
                                         AF.Sqrt)
                    with nc.allow_low_precision("rsqrt scale, values <= 1"):
                        nc.vector.reciprocal(ci[:, ca:ca + kch], c32[:, 0:kch])
                    nc.vector.tensor_tensor(
                        featp_s[:, ca:ca + kch, :],
                        featp_s[:, ca:ca + kch, :],
                        ci[:, ca:ca + kch].unsqueeze(2).to_broadcast(
                            [128, kch, D]),
                        OP.mult)

            # cj = rsqrt(max(in_deg, 1)) in fp16, prepared during the
            # scatter phase (DVE is idle then)
            cjT = bigpool.tile([D, NPAD], f16, tag="cjT")
            ind_s = bigpool.tile([D, NPAD], u8, tag="indT")
            nc.scalar.dma_start(out=ind_s[:], in_=indegTD)
            with tc.tile_pool(name="cjp", bufs=2) as cjpool:
                CJC = 1568
                for lo in range(0, NPAD, CJC):
                    hi2 = lo + CJC
                    t32 = cjpool.tile([D, CJC], f32, tag="t32")
                    nc.vector.tensor_scalar_max(t32[:], ind_s[:, lo:hi2], 1.0)
                    nc.scalar.activation(t32[:], t32[:], AF.Sqrt)
                    with nc.allow_low_precision("rsqrt, values <= 1"):
                        nc.vector.reciprocal(cjT[:, lo:hi2], t32[:])
            if iso:
                mask = bigpool.tile([D, NPAD], f16, tag="mask")
                nc.vector.tensor_scalar(mask[:], ind_s[:], 0.0, None,
                                        OP.is_gt)
                nc.vector.tensor_mul(cjT[:], cjT[:], mask[:])
                featT_s = bigpool.tile([D, NPAD], f32, tag="featT")
                nc.sync.dma_start(out=featT_s[:], in_=featTD)
                fbT = bigpool.tile([D, NPAD], f32, tag="fbT")
                nc.vector.tensor_scalar(mask[:], mask[:], -1.0, 1.0,
                                        OP.mult, OP.add)  # 1 - mask
                nc.vector.tensor_mul(fbT[:], featT_s[:], mask[:])

            # race-free scatter-adds: every batch has distinct dst rows and
            # consecutive batches hit different accumulators; the last few
            # avoid acc0 so its transpose readback overlaps the phase tail
            for i, (ca, L) in enumerate(plan):
                a = i % NACC
                if i >= len(plan) - 4:
                    a = 1 + (i % 2)
                K = -(-L // 128)
                gi = min(i // per, ngrp - 1)
                lo_b, hi_b, obase = bnd[gi]
                loc = sum(LL // 16 for _, LL in plan[lo_b:i])
                nc.gpsimd.dma_scatter_add(
                    accD[a][:, 0:D],
                    featp_s[:, ca:ca + K, :],
                    idx_tiles[gi][:, loc:loc + L // 16],
                    L,
                    L,
                    D,
                    elem_step=128,
                    queue_num=i % 2,
                    single_packet=False,
                )

            # ---- tail: read back, combine, cj scale, linear + relu -------
            hs = bigpool.tile([D, NPAD], f16, tag="hs")
            outT = bigpool.tile([D, NPAD], f16, tag="outT")
            with tc.tile_pool(name="htp", bufs=2) as htpool:
                ht0 = htpool.tile([128, NPAD], f16, tag="ht")
                nc.sync.dma_start(out=ht0[:], in_=accD[0], transpose=True)
                ht1 = htpool.tile([128, NPAD], f16, tag="ht")
                nc.sync.dma_start(out=ht1[:], in_=accD[1], transpose=True)
                nc.vector.tensor_tensor(hs[:], ht0[0:D, :], ht1[0:D, :],
                                        OP.add)
                ht2 = htpool.tile([128, NPAD], f16, tag="ht")
                nc.sync.dma_start(out=ht2[:], in_=accD[2], transpose=True)
                nc.vector.tensor_tensor(hs[:], hs[:], ht2[0:D, :], OP.add)
            if iso:
                nc.vector.tensor_mul(hs[:], hs[:], cjT[:])
                nc.vector.tensor_add(hs[:], hs[:], fbT[:])
            else:
                nc.vector.tensor_mul(hs[:], hs[:], cjT[:])

            CH = 512
            with tc.tile_pool(name="ps", bufs=2, space="PSUM") as pspool:
                for lo in range(0, NPAD, CH):
                    hi = min(lo + CH, NPAD)
                    po = pspool.tile([D, CH], f32, tag="po")
                    nc.tensor.matmul(po[:, 0:hi - lo], lhsT=wb_s[:],
                                     rhs=hs[:, lo:hi], start=True, stop=True)
                    nc.scalar.activation(outT[:, lo:hi], po[:, 0:hi - lo],
                                         AF.Relu, bias=bias_s[:, 0:1])
                    nc.sync.dma_start(out=outD[:, lo:hi], in_=outT[:, lo:hi])

    nc.compile()
    return nc


# ---------------------------------------------------------------------------
# Entry point
# ---------------------------------------------------------------------------

def kernel(features, src, dst, W, b):
    from concourse.bass_utils import run_bass_kernel_spmd

    per_core, meta = _host_prep(features, src, dst)

    key = (meta["plan"], meta["iso"], meta["tch2"])
    if key not in _CACHE:
        _CACHE[key] = _build_program(meta)
    nc = _CACHE[key]

    Wb = np.ascontiguousarray(np.asarray(W, np.float32).T.astype(np.float16))
    bias = np.ascontiguousarray(np.asarray(b, np.float32)[:, None])

    in_maps = []
    for c in range(NCORES):
        pc = per_core[c]
        m = {
            "featp": pc["featp"], "degp": pc["degp"],
            "idxcat": pc["idxcat"], "indegT": pc["indegT"],
            "wb": Wb, "bias": bias,
        }
        if meta["iso"]:
            m["featT"] = pc["featT"]
        in_maps.append(m)

    res = run_bass_kernel_spmd(nc, in_maps, core_ids=list(range(NCORES)))
    globals()["LAST_RESULTS"] = res
    out = np.concatenate(
        [res.results[c]["out"][:, :NPC].T for c in range(NCORES)], axis=0)
    return np.ascontiguousarray(out, dtype=np.float32)
